# revision 13
# baseline (speedup 1.0000x reference)
"""DiffOfGaussians Trainium2 kernel.

Math:
  out[b,u] = sum_{h,w,c} inputs[b,h,w,c] * F[h,w,u] + bias[u]
  F[h,w,u] = g(a1,s1) - g(a2,s1+s2),  g(a,s) = a*exp(-((w-ux)^2+(h-uy)^2)/(2s))/(2*pi*s)

The filter is separable: exp(-(dx^2+dy^2)/(2s)) = exp(-dx^2/2s)*exp(-dy^2/2s), so
  F[h,w,u] = Gx1[w,u]*gy1[u,h] + Gx2[w,u]*gy2[u,h]
with tiny 128x256 1-D tables (amplitudes folded into gy, the minus sign into gy2).

Sharding: H is split across the 8 cores (16 rows each). Each core reduces its
slab over C, transposes to put W on partitions, contracts over W on the tensor
engine (stationary = Gx block), then accumulates over its H rows with a fused
multiply-add using gy as a per-partition scalar. Host sums the 8 partial
(64,256) outputs (bias/8 is added on every core so the sum carries full bias).
"""

import sys

for _p in ("/opt/trn_rl_repo",):
    if _p not in sys.path:
        sys.path.insert(0, _p)

import numpy as np

import concourse.bass as bass
import concourse.tile as tile
from concourse import bacc, masks, mybir
from concourse.bass_utils import run_bass_kernel_spmd

F32 = mybir.dt.float32
AX = mybir.AxisListType
OP = mybir.AluOpType
AF = mybir.ActivationFunctionType

B, H, W, C, U = 64, 128, 128, 16, 256
NCORES = 8
HSH = H // NCORES  # 16 rows per core
INV2PI = float(1.0 / (2.0 * np.pi))

_CACHE = {}


def _build_kernel():
    nc = bacc.Bacc(
        "TRN2",
        target_bir_lowering=False,
        debug=False,
        num_devices=NCORES,
    )

    x_d = nc.dram_tensor("x", [B, HSH, W, C], F32, kind="ExternalInput").ap()
    yc_d = nc.dram_tensor("yc", [1, HSH], F32, kind="ExternalInput").ap()
    # packed params: col 2i+k = param i, units k*128..k*128+127
    # order: a1, a2, s1, s2, ux, uy, bias (cols 12:14), pad to 16
    prm_d = nc.dram_tensor("prm", [128, 16], F32, kind="ExternalInput").ap()
    # out[k, u_lo, b] = partial of out[b, k*128+u_lo]
    out_d = nc.dram_tensor("out", [2, 128, 64], F32, kind="ExternalOutput").ap()

    with tile.TileContext(nc) as tc:
        with (
            tc.tile_pool(name="singles", bufs=1) as singles,
            tc.tile_pool(name="gx", bufs=4) as gx_pool,
            tc.tile_pool(name="inp", bufs=8) as in_pool,
            tc.tile_pool(name="xr", bufs=4) as x_pool,
            tc.tile_pool(name="tree", bufs=2) as tree_pool,
            tc.tile_pool(name="ptr", bufs=2, space="PSUM") as tr_psum,
            tc.tile_pool(name="pmm", bufs=4, space="PSUM") as mm_psum,
        ):
            # ---------------- constants & parameters ----------------
            identity = singles.tile([128, 128], F32)
            masks.make_identity(nc, identity[:])

            # explicit zero-bias for ACT ops; the implicit bias=0.0 const
            # would be allocated outside Tile's pool tracking and race.
            zbias = singles.tile([128, 1], F32)
            nc.vector.memset(zbias[:], 0.0)

            iota_i = singles.tile([128, 128], mybir.dt.int32)
            nc.gpsimd.iota(iota_i[:], pattern=[[1, 128]], base=0, channel_multiplier=0)
            iota_f = singles.tile([128, 128], F32)
            nc.vector.tensor_copy(iota_f[:], iota_i[:])

            # single packed param DMA (8 tiny DMAs serialized badly in v2)
            prm_sb = singles.tile([128, 16], F32)
            nc.scalar.dma_start(out=prm_sb[:], in_=prm_d)
            _ord = ("a1", "a2", "s1", "s2", "ux", "uy")
            psb = {n: prm_sb[:, 2 * i : 2 * i + 2] for i, n in enumerate(_ord)}
            bias_sb = prm_sb[:, 12:14]

            yc_sb = singles.tile([128, HSH], F32)
            yc_bcast = bass.AP(
                tensor=yc_d.tensor, offset=yc_d.offset, ap=[[0, 128], [1, HSH]]
            )
            nc.gpsimd.dma_start(out=yc_sb[:], in_=yc_bcast)

            # derived per-unit params, all [128, 2]
            sig2 = singles.tile([128, 2], F32)
            nc.vector.tensor_add(sig2[:], psb["s1"], psb["s2"])
            rc1 = singles.tile([128, 2], F32)
            nc.vector.reciprocal(rc1[:], psb["s1"])
            rc2 = singles.tile([128, 2], F32)
            nc.vector.reciprocal(rc2[:], sig2[:])
            nis = []  # -1/(2 sigma_path)
            for p, rc in enumerate((rc1, rc2)):
                t = singles.tile([128, 2], F32, tag=f"nis{p}")
                nc.vector.tensor_scalar_mul(t[:], rc[:], -0.5)
                nis.append(t)
            # amplitude coefs: c1 = a1/(2 pi s1), c2n = -a2/(2 pi (s1+s2))
            coef = []
            for p, (a, rc, s) in enumerate(
                ((psb["a1"], rc1, INV2PI), (psb["a2"], rc2, -INV2PI))
            ):
                t0 = singles.tile([128, 2], F32, tag=f"coefa{p}")
                nc.vector.tensor_mul(t0[:], a, rc[:])
                t1 = singles.tile([128, 2], F32, tag=f"coef{p}")
                nc.vector.tensor_scalar_mul(t1[:], t0[:], s)
                coef.append(t1)

            # ---------------- Gx tables: Gxw[path][w, u] ----------------
            # dx2[k][u_lo, w] = (w - ux[u])^2   (shared across paths)
            dx2 = []
            for k in range(2):
                dx = gx_pool.tile([128, 128], F32, tag="dx")
                nc.vector.tensor_scalar(
                    dx[:], iota_f[:], psb["ux"][:, k : k + 1], None, op0=OP.subtract
                )
                d2 = singles.tile([128, 128], F32, tag=f"dx2_{k}")
                nc.scalar.activation(d2[:], dx[:], AF.Square, bias=zbias[:, 0:1])
                dx2.append(d2)

            gxw = []  # per path: [128(w), 256(u)]
            for p in range(2):
                t = singles.tile([128, 256], F32, tag=f"gxw{p}")
                gxw.append(t)
            for p in range(2):
                for k in range(2):
                    g = gx_pool.tile([128, 128], F32, tag="gx")
                    nc.scalar.activation(
                        g[:], dx2[k][:], AF.Exp,
                        bias=zbias[:, 0:1], scale=nis[p][:, k : k + 1],
                    )
                    ps = tr_psum.tile([128, 128], F32)
                    nc.tensor.transpose(ps[:], g[:], identity[:])
                    nc.scalar.copy(gxw[p][:, k * 128 : (k + 1) * 128], ps[:])

            # ---------------- gy tables: gy[path][k][u_lo, h] ----------------
            gy = [[None, None], [None, None]]
            for k in range(2):
                dy = gx_pool.tile([128, HSH], F32, tag="dy")
                nc.vector.tensor_scalar(
                    dy[:], yc_sb[:], psb["uy"][:, k : k + 1], None, op0=OP.subtract
                )
                dy2 = gx_pool.tile([128, HSH], F32, tag="dy2")
                nc.scalar.activation(dy2[:], dy[:], AF.Square, bias=zbias[:, 0:1])
                for p in range(2):
                    e = gx_pool.tile([128, HSH], F32, tag="gye")
                    nc.scalar.activation(
                        e[:], dy2[:], AF.Exp,
                        bias=zbias[:, 0:1], scale=nis[p][:, k : k + 1],
                    )
                    t = singles.tile([128, HSH], F32, tag=f"gy{p}_{k}")
                    nc.vector.tensor_scalar_mul(t[:], e[:], coef[p][:, k : k + 1])
                    gy[p][k] = t

            # ---------------- accumulators ----------------
            acc = []
            for k in range(2):
                t = singles.tile([128, 64], F32, tag=f"acc{k}")
                acc.append(t)
            nc.gpsimd.memset(acc[0][:], 0.0)
            nc.gpsimd.memset(acc[1][:], 0.0)

            # XT_all[w, h*64+b] = X[b, h, w] (c-reduced input, transposed)
            xt_all = singles.tile([128, HSH * 64], F32)

            # ---------------- main loop over h-pair tiles ----------------
            # c-reduce: tiles 0-1 fully on DVE (lowest latency, gate the
            # first matmul group); tiles 2+ fold c16->c8 on GpSimd first,
            # then a half-width DVE reduce. GpSimd cannot touch PSUM, so
            # the PSUM-reading scale-accumulate runs on DVE (ublk0) and
            # ACT-mult + GpSimd-add via SBUF bounce (ublk1).
            def gy_bcast(p, k, h0, nh):
                """[128, nh, 64] view of gy[p][k][:, h0:h0+nh] broadcast over b."""
                sl = gy[p][k][:, h0 : h0 + nh]
                return bass.AP(
                    tensor=sl.tensor, offset=sl.offset, ap=[*sl.ap, [0, 64]]
                )

            def mm_and_accum(h0, nh):
                """One matmul per (path, ublk) over nh h-rows, then a wide
                broadcast-multiply by gy (DVE), a contiguous in-place tree
                sum over h (GpSimd, SBUF only) and accumulation into acc."""
                n = nh * 64
                for p in range(2):
                    for k in range(2):
                        pmm = mm_psum.tile([128, 512], F32, tag="pmm")
                        nc.tensor.matmul(
                            pmm[:, :n],
                            gxw[p][:, k * 128 : (k + 1) * 128],
                            xt_all[:, h0 * 64 : h0 * 64 + n],
                            start=True,
                            stop=True,
                        )
                        tg = tree_pool.tile([128, 512], F32, tag="tg")
                        nc.vector.tensor_tensor(
                            tg[:, :n].rearrange("q (h b) -> q h b", b=64),
                            pmm[:, :n].rearrange("q (h b) -> q h b", b=64),
                            gy_bcast(p, k, h0, nh),
                            op=OP.mult,
                        )
                        m = n // 2
                        while m >= 64:
                            nc.gpsimd.tensor_add(
                                tg[:, :m], tg[:, :m], tg[:, m : 2 * m]
                            )
                            m //= 2
                        nc.gpsimd.tensor_add(acc[k][:], acc[k][:], tg[:, :64])

            for j in range(HSH // 2):
                t = in_pool.tile([128, W * C], F32, tag="t")
                xv = x_d.rearrange("b h w c -> h b (w c)")
                # one dma per h-row: outer dim 64 spreads across all 16 SDMA
                # engines (a [2,64,f] src splits by outer dim -> only 2 engines)
                for hh in range(2):
                    nc.sync.dma_start(
                        out=t[hh * 64 : (hh + 1) * 64, :], in_=xv[2 * j + hh]
                    )

                xr = x_pool.tile([128, 128], F32, tag="xr")
                tv = t.rearrange("p (w c) -> p w c", c=C)
                if 2 <= j <= 5:
                    # fold c16->c8 on GpSimd, half-width reduce on DVE
                    a = tree_pool.tile([128, W * 8], F32, tag="tr_a")
                    av = a.rearrange("p (w c) -> p w c", c=8)
                    nc.gpsimd.tensor_add(av[:], tv[:, :, 0:8], tv[:, :, 8:16])
                    nc.vector.reduce_sum(xr[:], av[:], axis=AX.X)
                else:
                    # first/last tiles: lowest-latency pure-DVE reduce
                    nc.vector.reduce_sum(xr[:], tv[:], axis=AX.X)

                ps = tr_psum.tile([128, 128], F32)
                nc.tensor.transpose(ps[:], xr[:], identity[:])
                nc.scalar.copy(xt_all[:, j * 128 : (j + 1) * 128], ps[:])

                if j == 3:
                    mm_and_accum(0, 8)
                elif j == 5:
                    mm_and_accum(8, 4)
                elif j == 7:
                    mm_and_accum(12, 4)

            # ---------------- bias (1/8 per core) and store ----------------
            bias8 = singles.tile([128, 2], F32)
            nc.vector.tensor_scalar_mul(bias8[:], bias_sb, 1.0 / NCORES)
            nc.gpsimd.tensor_scalar_add(acc[0][:], acc[0][:], bias8[:, 0:1])
            nc.gpsimd.tensor_scalar_add(acc[1][:], acc[1][:], bias8[:, 1:2])
            for k in range(2):
                nc.sync.dma_start(out=out_d[k], in_=acc[k][:])

    nc.compile()
    return nc


def _get_nc():
    if "nc" not in _CACHE:
        _CACHE["nc"] = _build_kernel()
    return _CACHE["nc"]


def pack_params(inputs: dict) -> np.ndarray:
    """[128, 16]: col 2i+k = param i (a1,a2,s1,s2,ux,uy,bias), unit block k."""
    prm = np.zeros((128, 16), dtype=np.float32)
    names = ("a1", "a2", "s1", "s2", "ux", "uy", "bias")
    for i, n in enumerate(names):
        v = np.asarray(inputs[n], dtype=np.float32).reshape(U)
        prm[:, 2 * i] = v[:128]
        prm[:, 2 * i + 1] = v[128:]
    return prm


def run(inputs: dict, trace: bool = False):
    """Run on 8 cores; returns (full_output, BassKernelResults)."""
    nc = _get_nc()
    x = np.ascontiguousarray(np.asarray(inputs["inputs"], dtype=np.float32))
    prm = pack_params(inputs)
    in_maps = []
    for i in range(NCORES):
        m = {
            "x": np.ascontiguousarray(x[:, i * HSH : (i + 1) * HSH]),
            "yc": np.arange(i * HSH, (i + 1) * HSH, dtype=np.float32).reshape(
                1, HSH
            ),
            "prm": prm,
        }
        in_maps.append(m)

    res = run_bass_kernel_spmd(
        nc, in_maps, core_ids=list(range(NCORES)), trace=trace
    )
    # partials: [2, 128, 64] -> out[b, k*128+u_lo]
    total = np.zeros((2, 128, 64), dtype=np.float64)
    for r in res.results:
        total += r["out"].astype(np.float64)
    out = total.transpose(2, 0, 1).reshape(64, 256).astype(np.float32)
    return out, res


def kernel(**inputs) -> np.ndarray:
    out, _ = run(inputs, trace=False)
    return out


# revision 14
# speedup vs baseline: 1.0821x; 1.0821x over previous
"""DiffOfGaussians Trainium2 kernel.

Math:
  out[b,u] = sum_{h,w,c} inputs[b,h,w,c] * F[h,w,u] + bias[u]
  F[h,w,u] = g(a1,s1) - g(a2,s1+s2),  g(a,s) = a*exp(-((w-ux)^2+(h-uy)^2)/(2s))/(2*pi*s)

The filter is separable: exp(-(dx^2+dy^2)/(2s)) = exp(-dx^2/2s)*exp(-dy^2/2s), so
  F[h,w,u] = Gx1[w,u]*gy1[u,h] + Gx2[w,u]*gy2[u,h]
with tiny 128x256 1-D tables (amplitudes folded into gy, the minus sign into gy2).

Sharding: H is split across the 8 cores (16 rows each). Each core reduces its
slab over C, transposes to put W on partitions, contracts over W on the tensor
engine (stationary = Gx block), then accumulates over its H rows with a fused
multiply-add using gy as a per-partition scalar. Host sums the 8 partial
(64,256) outputs (bias/8 is added on every core so the sum carries full bias).
"""

import sys

for _p in ("/opt/trn_rl_repo",):
    if _p not in sys.path:
        sys.path.insert(0, _p)

import numpy as np

import concourse.bass as bass
import concourse.tile as tile
from concourse import bacc, masks, mybir
from concourse.bass_utils import run_bass_kernel_spmd

F32 = mybir.dt.float32
AX = mybir.AxisListType
OP = mybir.AluOpType
AF = mybir.ActivationFunctionType

B, H, W, C, U = 64, 128, 128, 16, 256
NCORES = 8
HSH = H // NCORES  # 16 rows per core
INV2PI = float(1.0 / (2.0 * np.pi))

_CACHE = {}


def _build_kernel():
    nc = bacc.Bacc(
        "TRN2",
        target_bir_lowering=False,
        debug=False,
        num_devices=NCORES,
    )

    x_d = nc.dram_tensor("x", [B, HSH, W, C], F32, kind="ExternalInput").ap()
    yc_d = nc.dram_tensor("yc", [1, HSH], F32, kind="ExternalInput").ap()
    # packed params: col 2i+k = param i, units k*128..k*128+127
    # order: a1, a2, s1, s2, ux, uy, bias (cols 12:14), pad to 16
    prm_d = nc.dram_tensor("prm", [128, 16], F32, kind="ExternalInput").ap()
    # out[k, u_lo, b] = partial of out[b, k*128+u_lo]
    out_d = nc.dram_tensor("out", [2, 128, 64], F32, kind="ExternalOutput").ap()

    with tile.TileContext(nc) as tc:
        with (
            tc.tile_pool(name="singles", bufs=1) as singles,
            tc.tile_pool(name="gx", bufs=4) as gx_pool,
            tc.tile_pool(name="inp", bufs=8) as in_pool,
            tc.tile_pool(name="xr", bufs=4) as x_pool,
            tc.tile_pool(name="tree", bufs=2) as tree_pool,
            tc.tile_pool(name="ptr", bufs=2, space="PSUM") as tr_psum,
            tc.tile_pool(name="pmm", bufs=6, space="PSUM") as mm_psum,
        ):
            # ---------------- constants & parameters ----------------
            identity = singles.tile([128, 128], F32)
            masks.make_identity(nc, identity[:])

            # explicit zero-bias for ACT ops; the implicit bias=0.0 const
            # would be allocated outside Tile's pool tracking and race.
            zbias = singles.tile([128, 1], F32)
            nc.vector.memset(zbias[:], 0.0)

            iota_i = singles.tile([128, 128], mybir.dt.int32)
            nc.gpsimd.iota(iota_i[:], pattern=[[1, 128]], base=0, channel_multiplier=0)
            iota_f = singles.tile([128, 128], F32)
            nc.vector.tensor_copy(iota_f[:], iota_i[:])

            # single packed param DMA (8 tiny DMAs serialized badly in v2)
            prm_sb = singles.tile([128, 16], F32)
            nc.scalar.dma_start(out=prm_sb[:], in_=prm_d)
            _ord = ("a1", "a2", "s1", "s2", "ux", "uy")
            psb = {n: prm_sb[:, 2 * i : 2 * i + 2] for i, n in enumerate(_ord)}
            bias_sb = prm_sb[:, 12:14]

            yc_sb = singles.tile([128, HSH], F32)
            yc_bcast = bass.AP(
                tensor=yc_d.tensor, offset=yc_d.offset, ap=[[0, 128], [1, HSH]]
            )
            nc.gpsimd.dma_start(out=yc_sb[:], in_=yc_bcast)

            # derived per-unit params, all [128, 2]
            sig2 = singles.tile([128, 2], F32)
            nc.vector.tensor_add(sig2[:], psb["s1"], psb["s2"])
            rc1 = singles.tile([128, 2], F32)
            nc.vector.reciprocal(rc1[:], psb["s1"])
            rc2 = singles.tile([128, 2], F32)
            nc.vector.reciprocal(rc2[:], sig2[:])
            nis = []  # -1/(2 sigma_path)
            for p, rc in enumerate((rc1, rc2)):
                t = singles.tile([128, 2], F32, tag=f"nis{p}")
                nc.vector.tensor_scalar_mul(t[:], rc[:], -0.5)
                nis.append(t)
            # amplitude coefs: c1 = a1/(2 pi s1), c2n = -a2/(2 pi (s1+s2))
            coef = []
            for p, (a, rc, s) in enumerate(
                ((psb["a1"], rc1, INV2PI), (psb["a2"], rc2, -INV2PI))
            ):
                t0 = singles.tile([128, 2], F32, tag=f"coefa{p}")
                nc.vector.tensor_mul(t0[:], a, rc[:])
                t1 = singles.tile([128, 2], F32, tag=f"coef{p}")
                nc.vector.tensor_scalar_mul(t1[:], t0[:], s)
                coef.append(t1)

            # ---------------- Gx tables: Gxw[path][w, u] ----------------
            # dx2[k][u_lo, w] = (w - ux[u])^2   (shared across paths)
            dx2 = []
            for k in range(2):
                dx = gx_pool.tile([128, 128], F32, tag="dx")
                nc.vector.tensor_scalar(
                    dx[:], iota_f[:], psb["ux"][:, k : k + 1], None, op0=OP.subtract
                )
                d2 = singles.tile([128, 128], F32, tag=f"dx2_{k}")
                nc.scalar.activation(d2[:], dx[:], AF.Square, bias=zbias[:, 0:1])
                dx2.append(d2)

            gxw = []  # per path: [128(w), 256(u)]
            for p in range(2):
                t = singles.tile([128, 256], F32, tag=f"gxw{p}")
                gxw.append(t)
            for p in range(2):
                for k in range(2):
                    g = gx_pool.tile([128, 128], F32, tag="gx")
                    nc.scalar.activation(
                        g[:], dx2[k][:], AF.Exp,
                        bias=zbias[:, 0:1], scale=nis[p][:, k : k + 1],
                    )
                    ps = tr_psum.tile([128, 128], F32)
                    nc.tensor.transpose(ps[:], g[:], identity[:])
                    nc.scalar.copy(gxw[p][:, k * 128 : (k + 1) * 128], ps[:])

            # ---------------- gy tables: gy[path][k][u_lo, h] ----------------
            gy = [[None, None], [None, None]]
            for k in range(2):
                dy = gx_pool.tile([128, HSH], F32, tag="dy")
                nc.vector.tensor_scalar(
                    dy[:], yc_sb[:], psb["uy"][:, k : k + 1], None, op0=OP.subtract
                )
                dy2 = gx_pool.tile([128, HSH], F32, tag="dy2")
                nc.scalar.activation(dy2[:], dy[:], AF.Square, bias=zbias[:, 0:1])
                for p in range(2):
                    e = gx_pool.tile([128, HSH], F32, tag="gye")
                    nc.scalar.activation(
                        e[:], dy2[:], AF.Exp,
                        bias=zbias[:, 0:1], scale=nis[p][:, k : k + 1],
                    )
                    t = singles.tile([128, HSH], F32, tag=f"gy{p}_{k}")
                    nc.vector.tensor_scalar_mul(t[:], e[:], coef[p][:, k : k + 1])
                    gy[p][k] = t

            # ---------------- accumulators ----------------
            acc = []
            for k in range(2):
                t = singles.tile([128, 64], F32, tag=f"acc{k}")
                acc.append(t)
            nc.gpsimd.memset(acc[0][:], 0.0)
            nc.gpsimd.memset(acc[1][:], 0.0)

            # XT_all[w, h*64+b] = X[b, h, w] (c-reduced input, transposed)
            xt_all = singles.tile([128, HSH * 64], F32)

            # ---------------- main loop over h-pair tiles ----------------
            # c-reduce: tiles 0-1 fully on DVE (lowest latency, gate the
            # first matmul group); tiles 2+ fold c16->c8 on GpSimd first,
            # then a half-width DVE reduce. GpSimd cannot touch PSUM, so
            # the PSUM-reading scale-accumulate runs on DVE (ublk0) and
            # ACT-mult + GpSimd-add via SBUF bounce (ublk1).
            def gy_bcast(p, k, h0, nh):
                """[128, nh, 64] view of gy[p][k][:, h0:h0+nh] broadcast over b."""
                sl = gy[p][k][:, h0 : h0 + nh]
                return bass.AP(
                    tensor=sl.tensor, offset=sl.offset, ap=[*sl.ap, [0, 64]]
                )

            # matmul groups: (h0, nh); grp0 gets one PSUM tile per (p,k),
            # grp1/2 pack both k into one tile (8 PSUM banks total).
            groups = [(0, 8), (8, 4), (12, 4)]
            pmm_tiles = {}

            def mm_group(gi):
                h0, nh = groups[gi]
                n = nh * 64
                for p in range(2):
                    if nh == 4:
                        tile_pk = mm_psum.tile([128, 512], F32, tag="pmm")
                        for k in range(2):
                            pmm_tiles[(gi, p, k)] = tile_pk[:, k * n : (k + 1) * n]
                    for k in range(2):
                        if nh == 8:
                            t_ = mm_psum.tile([128, 512], F32, tag="pmm")
                            pmm_tiles[(gi, p, k)] = t_[:]
                        nc.tensor.matmul(
                            pmm_tiles[(gi, p, k)],
                            gxw[p][:, k * 128 : (k + 1) * 128],
                            xt_all[:, h0 * 64 : h0 * 64 + n],
                            start=True,
                            stop=True,
                        )

            def accum_group(gi):
                h0, nh = groups[gi]
                n = nh * 64
                for p in range(2):
                    for k in range(2):
                        pmm = pmm_tiles[(gi, p, k)]
                        tg = tree_pool.tile([128, 512], F32, tag="tg")
                        nc.vector.tensor_tensor(
                            tg[:, :n].rearrange("q (h b) -> q h b", b=64),
                            pmm.rearrange("q (h b) -> q h b", b=64),
                            gy_bcast(p, k, h0, nh),
                            op=OP.mult,
                        )
                        m = n // 2
                        while m >= 64:
                            nc.gpsimd.tensor_add(
                                tg[:, :m], tg[:, :m], tg[:, m : 2 * m]
                            )
                            m //= 2
                        nc.gpsimd.tensor_add(acc[k][:], acc[k][:], tg[:, :64])

            for j in range(HSH // 2):
                t = in_pool.tile([128, W * C], F32, tag="t")
                xv = x_d.rearrange("b h w c -> h b (w c)")
                # one dma per h-row: outer dim 64 spreads across all 16 SDMA
                # engines (a [2,64,f] src splits by outer dim -> only 2 engines)
                for hh in range(2):
                    nc.sync.dma_start(
                        out=t[hh * 64 : (hh + 1) * 64, :], in_=xv[2 * j + hh]
                    )

                xr = x_pool.tile([128, 128], F32, tag="xr")
                tv = t.rearrange("p (w c) -> p w c", c=C)
                if 2 <= j <= 4:
                    # fold c16->c8 on GpSimd, half-width reduce on DVE.
                    # First/last tiles stay pure-DVE: lower latency, and the
                    # last folds would queue behind tree work on GpSimd.
                    a = tree_pool.tile([128, W * 8], F32, tag="tr_a")
                    av = a.rearrange("p (w c) -> p w c", c=8)
                    nc.gpsimd.tensor_add(av[:], tv[:, :, 0:8], tv[:, :, 8:16])
                    nc.vector.reduce_sum(xr[:], av[:], axis=AX.X)
                else:
                    nc.vector.reduce_sum(xr[:], tv[:], axis=AX.X)

                ps = tr_psum.tile([128, 128], F32)
                nc.tensor.transpose(ps[:], xr[:], identity[:])
                nc.scalar.copy(xt_all[:, j * 128 : (j + 1) * 128], ps[:])

                # matmuls interleave with transposes in the PE stream (real
                # deps only); all DVE/GpSimd accumulation is emitted after
                # the reduces so it cannot head-of-line block them.
                if j == 3:
                    mm_group(0)
                elif j == 5:
                    mm_group(1)
                elif j == 7:
                    mm_group(2)

            for gi in range(3):
                accum_group(gi)

            # ---------------- bias (1/8 per core) and store ----------------
            bias8 = singles.tile([128, 2], F32)
            nc.vector.tensor_scalar_mul(bias8[:], bias_sb, 1.0 / NCORES)
            nc.vector.tensor_scalar_add(acc[0][:], acc[0][:], bias8[:, 0:1])
            nc.vector.tensor_scalar_add(acc[1][:], acc[1][:], bias8[:, 1:2])
            for k in range(2):
                nc.sync.dma_start(out=out_d[k], in_=acc[k][:])

    nc.compile()
    return nc


def _get_nc():
    if "nc" not in _CACHE:
        _CACHE["nc"] = _build_kernel()
    return _CACHE["nc"]


def pack_params(inputs: dict) -> np.ndarray:
    """[128, 16]: col 2i+k = param i (a1,a2,s1,s2,ux,uy,bias), unit block k."""
    prm = np.zeros((128, 16), dtype=np.float32)
    names = ("a1", "a2", "s1", "s2", "ux", "uy", "bias")
    for i, n in enumerate(names):
        v = np.asarray(inputs[n], dtype=np.float32).reshape(U)
        prm[:, 2 * i] = v[:128]
        prm[:, 2 * i + 1] = v[128:]
    return prm


def run(inputs: dict, trace: bool = False):
    """Run on 8 cores; returns (full_output, BassKernelResults)."""
    nc = _get_nc()
    x = np.ascontiguousarray(np.asarray(inputs["inputs"], dtype=np.float32))
    prm = pack_params(inputs)
    in_maps = []
    for i in range(NCORES):
        m = {
            "x": np.ascontiguousarray(x[:, i * HSH : (i + 1) * HSH]),
            "yc": np.arange(i * HSH, (i + 1) * HSH, dtype=np.float32).reshape(
                1, HSH
            ),
            "prm": prm,
        }
        in_maps.append(m)

    res = run_bass_kernel_spmd(
        nc, in_maps, core_ids=list(range(NCORES)), trace=trace
    )
    # partials: [2, 128, 64] -> out[b, k*128+u_lo]
    total = np.zeros((2, 128, 64), dtype=np.float64)
    for r in res.results:
        total += r["out"].astype(np.float64)
    out = total.transpose(2, 0, 1).reshape(64, 256).astype(np.float32)
    return out, res


def kernel(**inputs) -> np.ndarray:
    out, _ = run(inputs, trace=False)
    return out
